# revision 16
# baseline (speedup 1.0000x reference)
"""Trainium2 Bass kernel for nn_Attention_41755672052568.

Self-attention block on x:(16,512,32,32):
  GroupNorm(32,eps=1e-6,affine) -> q,k,v = 1x1 convs -> softmax(q^T k / sqrt(C))
  -> out = attn @ v -> 1x1 conv proj -> + residual

Strategy: data-parallel over batch B=16 across 8 NeuronCores (2 samples/core).
Per sample everything is kept on-chip:
  - GroupNorm stats via bn_stats/bn_aggr + tiny mask-matmuls for the
    cross-partition group reduce/expand; normalize+cast to bf16 on DVE.
    Both samples' stats run in one fused phase so the ACT Ln/Exp table set
    is loaded exactly once each for the whole kernel.
  - All GEMMs in bf16 with fp32 PSUM accumulation.
  - Attention is computed transposed (S = E^T = k^T q laid out [j, i]) so the
    second bmm needs no transposes; softmax skips the max-subtraction
    (|E| <= ~7 for randn inputs) and normalization happens after the O GEMM
    using a replicated row-sum computed with a ones-matmul and a fast DVE
    reciprocal (reciprocal_approx_fast, ~51 ULP).
  - v bias and gn affine are folded into weights/biases on the host
    (1x1 convs are linear), the softmax scale is folded into wq.
"""

import numpy as np
import ml_dtypes

B, C, HW = 16, 512, 1024
NCORES = 8
SPC = B // NCORES  # samples per core
P = 128
CT = C // P        # channel tiles (4)
JT = HW // P       # j tiles (8)
NH = HW // 512     # free-dim halves (2)
GS = 16            # channels per group (512/32)
GPT = P // GS      # groups per channel-tile (8)
U = SPC * CT       # channel-tile units across both samples (8)
EPS = 1e-6

_CACHE = {}


def _emit_consts(nc, tc, const, dram, mybir):
    f32 = mybir.dt.float32
    bf16 = mybir.dt.bfloat16
    t = {}
    t["wq_sb"] = const.tile([P, CT, C], bf16, name="wq_sb")
    t["wk_sb"] = const.tile([P, CT, C], bf16, name="wk_sb")
    t["wv_sb"] = const.tile([P, CT, C], bf16, name="wv_sb")
    t["wp_sb"] = const.tile([P, CT, C], bf16, name="wp_sb")
    for sb, name in ((t["wq_sb"], "wqT"), (t["wk_sb"], "wkT"),
                     (t["wv_sb"], "wvT"), (t["wp_sb"], "wpT")):
        nc.sync.dma_start(
            sb[:], dram[name].ap().rearrange("(t p) c -> p t c", p=P))
    t["bqk_sb"] = const.tile([P, 2 * CT], f32, name="bqk_sb")
    nc.sync.dma_start(t["bqk_sb"][:], dram["bqk"].ap())
    t["bp_sb"] = const.tile([P, CT], f32, name="bp_sb")
    nc.sync.dma_start(t["bp_sb"][:], dram["bp"].ap())
    t["gmask_sb"] = const.tile([P, GPT], f32, name="gmask_sb")
    nc.sync.dma_start(t["gmask_sb"][:], dram["gmask"].ap())
    t["gexp_sb"] = const.tile([P, P], f32, name="gexp_sb")
    nc.sync.dma_start(t["gexp_sb"][:], dram["gexpand"].ap())
    t["ones_sb"] = const.tile([P, P], bf16, name="ones_sb")
    nc.vector.memset(t["ones_sb"][:], 1.0)
    t["eps_sb"] = const.tile([P, 1], f32, name="eps_sb")
    nc.vector.memset(t["eps_sb"][:], EPS)
    t["zero_sb"] = const.tile([P, 1], f32, name="zero_sb")
    nc.vector.memset(t["zero_sb"][:], 0.0)
    return t


def _emit_body(nc, tc, pools, cst, dram, mybir):
    """One full pass over this core's SPC samples."""
    f32 = mybir.dt.float32
    bf16 = mybir.dt.bfloat16
    AF = mybir.ActivationFunctionType
    OP = mybir.AluOpType

    (xpool, xnpool, qkpool, vtpool, atpool, rpool, onpool, outpool, stats,
     psum) = pools

    x_in = dram["x"]
    out_dram = dram["out"]

    # ---------- phase A helper: load + groupnorm for one sample ----------
    # Sample s+1's phase A is emitted after sample s's GEMMs so its DVE/ACT
    # work fills idle slots instead of delaying PSUM evacuations.
    x_sbs = []
    xn_sbs = []

    def emit_phase_a(s):
        x_sb = xpool.tile([P, CT, HW], f32, tag=f"x{s}")
        nc.sync.dma_start(
            x_sb[:], x_in.ap()[s].rearrange("(t p) j -> p t j", p=P))
        x_sbs.append(x_sb)

        # per-channel mean / E[x^2]
        stats_all = stats.tile([P, CT, 2], f32, tag="stats_all")
        for t in range(CT):
            bnst = stats.tile([P, 2, 6], f32, tag="bnst")
            xin = x_sb[:, t, :].rearrange("p (c f) -> p c f", f=512)
            for h in range(2):
                nc.vector.bn_stats(out=bnst[:, h, :], in_=xin[:, h, :])
            nc.vector.bn_aggr(out=stats_all[:, t, :], in_=bnst[:])
        m2 = stats.tile([P, CT], f32, tag="m2")
        nc.vector.tensor_tensor(
            m2[:], stats_all[:, :, 0], stats_all[:, :, 0], OP.mult)
        nc.vector.tensor_tensor(
            stats_all[:, :, 1], stats_all[:, :, 1], m2[:], OP.add)
        # group-average across partitions: [8, CT, 2] = (mean_g, Ex2_g)
        gps = psum.tile([GPT, CT, 2], f32, tag="ps")
        nc.tensor.matmul(gps[:], cst["gmask_sb"][:], stats_all[:],
                         start=True, stop=True)
        gsb = stats.tile([GPT, CT, 2], f32, tag="gsb")
        nc.vector.tensor_copy(gsb[:], gps[:])
        gm2 = stats.tile([GPT, CT], f32, tag="gm2")
        nc.vector.tensor_tensor(gm2[:], gsb[:, :, 0], gsb[:, :, 0], OP.mult)
        varg = stats.tile([GPT, CT], f32, tag="varg")
        nc.vector.tensor_tensor(varg[:], gsb[:, :, 1], gm2[:], OP.subtract)
        # s_g = rsqrt(var+eps) = exp(-0.5*ln(var+eps));  mus_g = mean_g*s_g
        lnv = stats.tile([GPT, CT], f32, tag="lnv")
        nc.scalar.activation(lnv[:], varg[:], AF.Ln,
                             bias=cst["eps_sb"][0:GPT, :], scale=1.0)
        smus = stats.tile([P, 2 * CT], f32, tag="smus")
        nc.vector.memset(smus[:], 0.0)
        nc.scalar.activation(smus[0:GPT, 0:CT], lnv[:], AF.Exp,
                             bias=cst["zero_sb"][0:GPT, :], scale=-0.5)
        nc.vector.tensor_tensor(
            smus[0:GPT, CT:2 * CT], gsb[:, :, 0], smus[0:GPT, 0:CT], OP.mult)
        # expand group -> channel: chan[p, t]=s, chan[p, CT+t]=mu*s
        cps = psum.tile([P, 2 * CT], f32, tag="ps")
        nc.tensor.matmul(cps[:], cst["gexp_sb"][:], smus[:],
                         start=True, stop=True)
        chan = stats.tile([P, 2 * CT], f32, tag="chan")
        nc.vector.tensor_copy(chan[:], cps[:])

        # normalize + cast to bf16: xn = x*s - mu*s
        xn_sb = xnpool.tile([P, CT, HW], bf16, tag=f"xn{s}")
        for t in range(CT):
            nc.vector.tensor_scalar(
                out=xn_sb[:, t, :], in0=x_sb[:, t, :],
                scalar1=chan[:, t:t + 1], scalar2=chan[:, CT + t:CT + t + 1],
                op0=OP.mult, op1=OP.subtract)
        xn_sbs.append(xn_sb)

    # ---------- phase B: attention per sample ----------
    for s in range(SPC):
        emit_phase_a(s)
    for s in range(SPC):
        x_sb = x_sbs[s]
        xn_sb = xn_sbs[s]

        # Q, K GEMMs (softmax scale folded into wq)
        q_sb = qkpool.tile([P, CT, HW], bf16, tag="q")
        k_sb = qkpool.tile([P, CT, HW], bf16, tag="k")
        for dst, w_sb, boff in ((q_sb, cst["wq_sb"], 0),
                                (k_sb, cst["wk_sb"], CT)):
            for m in range(CT):
                ps = psum.tile([P, HW], f32, tag="ps")
                for n in range(NH):
                    for kt in range(CT):
                        nc.tensor.matmul(
                            ps[:, n * 512:(n + 1) * 512],
                            w_sb[:, kt, m * P:(m + 1) * P],
                            xn_sb[:, kt, n * 512:(n + 1) * 512],
                            start=(kt == 0), stop=(kt == CT - 1))
                nc.scalar.activation(
                    dst[:, m, :], ps[:], AF.Identity,
                    bias=cst["bqk_sb"][:, boff + m:boff + m + 1], scale=1.0)

        # vT GEMM: vT[j, c] = xn^T @ wv^T (bias folded into bp)
        vt_sb = vtpool.tile([P, JT, C], bf16, tag="vt")
        for mjp in range(JT // 2):
            ps = psum.tile([P, HW], f32, tag="ps")
            psv = ps[:].rearrange("p (h c) -> p h c", h=2)
            for h in range(2):
                mj = 2 * mjp + h
                for kt in range(CT):
                    nc.tensor.matmul(
                        psv[:, h, :], xn_sb[:, kt, mj * P:(mj + 1) * P],
                        cst["wv_sb"][:, kt, :],
                        start=(kt == 0), stop=(kt == CT - 1))
            nc.vector.tensor_copy(vt_sb[:, 2 * mjp:2 * mjp + 2, :], psv[:])

        # S = E^T GEMM + exp (no max subtraction; |E| <= ~7)
        at_sb = atpool.tile([P, JT, HW], bf16, tag="at")
        for mj in range(JT):
            ps = psum.tile([P, HW], f32, tag="ps")
            for n in range(NH):
                for kt in range(CT):
                    nc.tensor.matmul(
                        ps[:, n * 512:(n + 1) * 512],
                        k_sb[:, kt, mj * P:(mj + 1) * P],
                        q_sb[:, kt, n * 512:(n + 1) * 512],
                        start=(kt == 0), stop=(kt == CT - 1))
            nc.scalar.activation(at_sb[:, mj, :], ps[:], AF.Exp,
                                 bias=cst["zero_sb"][:])

        # row sums r_i replicated over partitions; rinv = 1/r on DVE
        rinv_sb = rpool.tile([P, HW], f32, tag="rinv")
        ps = psum.tile([P, HW], f32, tag="ps")
        for n in range(NH):
            for mj in range(JT):
                nc.tensor.matmul(
                    ps[:, n * 512:(n + 1) * 512], cst["ones_sb"][:],
                    at_sb[:, mj, n * 512:(n + 1) * 512],
                    start=(mj == 0), stop=(mj == JT - 1))
        nc.vector.reciprocal_approx_fast(out=rinv_sb[:], in_=ps[:])

        # O GEMM + normalize
        on_sb = onpool.tile([P, CT, HW], bf16, tag="on")
        for mc in range(CT):
            ps = psum.tile([P, HW], f32, tag="ps")
            for n in range(NH):
                for kj in range(JT):
                    nc.tensor.matmul(
                        ps[:, n * 512:(n + 1) * 512],
                        vt_sb[:, kj, mc * P:(mc + 1) * P],
                        at_sb[:, kj, n * 512:(n + 1) * 512],
                        start=(kj == 0), stop=(kj == JT - 1))
            nc.vector.tensor_tensor(
                on_sb[:, mc, :], ps[:], rinv_sb[:], OP.mult)

        # proj GEMM + bias + residual
        out_sb = outpool.tile([P, CT, HW], f32, tag="out")
        for m in range(CT):
            ps = psum.tile([P, HW], f32, tag="ps")
            for n in range(NH):
                for kt in range(CT):
                    nc.tensor.matmul(
                        ps[:, n * 512:(n + 1) * 512],
                        cst["wp_sb"][:, kt, m * P:(m + 1) * P],
                        on_sb[:, kt, n * 512:(n + 1) * 512],
                        start=(kt == 0), stop=(kt == CT - 1))
            nc.vector.scalar_tensor_tensor(
                out_sb[:, m, :], ps[:], cst["bp_sb"][:, m:m + 1],
                x_sb[:, m, :], OP.add, OP.add)
        nc.sync.dma_start(
            out_dram.ap()[s].rearrange("(t p) j -> p t j", p=P), out_sb[:])


def _build_nc(loop_reps=None):
    import concourse.bacc as bacc
    import concourse.tile as tile
    import concourse.mybir as mybir

    f32 = mybir.dt.float32
    bf16 = mybir.dt.bfloat16

    nc = bacc.Bacc("TRN2", target_bir_lowering=False, debug=False,
                   num_devices=NCORES)

    dram = {
        "x": nc.dram_tensor("x", [SPC, C, HW], f32, kind="ExternalInput"),
        "wqT": nc.dram_tensor("wqT", [C, C], bf16, kind="ExternalInput"),
        "wkT": nc.dram_tensor("wkT", [C, C], bf16, kind="ExternalInput"),
        "wvT": nc.dram_tensor("wvT", [C, C], bf16, kind="ExternalInput"),
        "wpT": nc.dram_tensor("wpT", [C, C], bf16, kind="ExternalInput"),
        "bqk": nc.dram_tensor("bqk", [P, 2 * CT], f32, kind="ExternalInput"),
        "bp": nc.dram_tensor("bp", [P, CT], f32, kind="ExternalInput"),
        "gmask": nc.dram_tensor("gmask", [P, GPT], f32, kind="ExternalInput"),
        "gexpand": nc.dram_tensor("gexpand", [P, P], f32,
                                  kind="ExternalInput"),
        "out": nc.dram_tensor("out", [SPC, C, HW], f32,
                              kind="ExternalOutput"),
    }

    from contextlib import ExitStack

    with tile.TileContext(nc) as tc:
        with ExitStack() as ctx:
            const = ctx.enter_context(tc.tile_pool(name="const", bufs=1))
            pools = (
                ctx.enter_context(tc.tile_pool(name="xp", bufs=1)),
                ctx.enter_context(tc.tile_pool(name="xnp", bufs=1)),
                ctx.enter_context(tc.tile_pool(name="qkp", bufs=1)),
                ctx.enter_context(tc.tile_pool(name="vtp", bufs=1)),
                ctx.enter_context(tc.tile_pool(name="atp", bufs=1)),
                ctx.enter_context(tc.tile_pool(name="rp", bufs=2)),
                ctx.enter_context(tc.tile_pool(name="onp", bufs=1)),
                ctx.enter_context(tc.tile_pool(name="outp", bufs=2)),
                ctx.enter_context(tc.tile_pool(name="stats", bufs=2)),
                ctx.enter_context(tc.tile_pool(name="psum", bufs=4,
                                               space="PSUM")),
            )
            cst = _emit_consts(nc, tc, const, dram, mybir)
            if loop_reps is None:
                _emit_body(nc, tc, pools, cst, dram, mybir)
            else:
                with tc.For_i(0, loop_reps, 1):
                    _emit_body(nc, tc, pools, cst, dram, mybir)

    nc.compile()
    return nc


def get_nc(loop_reps=None):
    key = ("nc", loop_reps)
    if key not in _CACHE:
        _CACHE[key] = _build_nc(loop_reps)
    return _CACHE[key]


def make_in_maps(x, gn_gamma, gn_beta, wq, bq, wk, bk, wv, bv, wp, bp):
    x = np.asarray(x, np.float32).reshape(B, C, HW)
    gamma = np.asarray(gn_gamma, np.float64)
    beta = np.asarray(gn_beta, np.float64)
    wq = np.asarray(wq, np.float64)
    wk = np.asarray(wk, np.float64)
    wv = np.asarray(wv, np.float64)
    wp = np.asarray(wp, np.float64)
    bq = np.asarray(bq, np.float64)
    bk = np.asarray(bk, np.float64)
    bv = np.asarray(bv, np.float64)
    bp = np.asarray(bp, np.float64)

    scale = C ** -0.5
    wq_eff = (wq * gamma[None, :]) * scale
    bq_eff = (wq @ beta + bq) * scale
    wk_eff = wk * gamma[None, :]
    bk_eff = wk @ beta + bk
    wv_eff = wv * gamma[None, :]
    bv_eff = wv @ beta + bv
    bp_eff = wp @ bv_eff + bp

    bf = ml_dtypes.bfloat16
    wqT = np.ascontiguousarray(wq_eff.T).astype(bf)
    wkT = np.ascontiguousarray(wk_eff.T).astype(bf)
    wvT = np.ascontiguousarray(wv_eff.T).astype(bf)
    wpT = np.ascontiguousarray(wp.T).astype(bf)
    bqk = np.ascontiguousarray(
        np.concatenate([bq_eff.reshape(CT, P).T, bk_eff.reshape(CT, P).T],
                       axis=1)).astype(np.float32)
    bpp = np.ascontiguousarray(bp_eff.reshape(CT, P).T).astype(np.float32)

    gmask = np.zeros((P, GPT), np.float32)
    for p_ in range(P):
        gmask[p_, p_ // GS] = 1.0 / GS
    gexpand = np.zeros((P, P), np.float32)
    for p_ in range(P):
        gexpand[p_ // GS, p_] = 1.0

    in_maps = []
    for c in range(NCORES):
        in_maps.append({
            "x": np.ascontiguousarray(x[c * SPC:(c + 1) * SPC]),
            "wqT": wqT, "wkT": wkT, "wvT": wvT, "wpT": wpT,
            "bqk": bqk, "bp": bpp, "gmask": gmask, "gexpand": gexpand,
        })
    return in_maps


def kernel(**inputs):
    from concourse.bass_utils import run_bass_kernel_spmd

    nc = get_nc()
    in_maps = make_in_maps(**inputs)
    res = run_bass_kernel_spmd(nc, in_maps, core_ids=list(range(NCORES)))
    out = np.concatenate([r["out"] for r in res.results], axis=0)
    return np.ascontiguousarray(out.reshape(B, C, 32, 32), dtype=np.float32)
